# revision 8
# baseline (speedup 1.0000x reference)
"""BRGCN (2-layer relational GCN with bilinear attention) on 8 TRN2 NeuronCores.

Sharding: edges are sharded by dst node (6250 nodes/core) so the
(relation, dst) segment softmax and the z scatter are fully core-local; node
features are replicated. Per core the edges are sorted by t = dst_local*8+rel,
binned into 391 blocks of 128 t-rows and padded into 128-edge K-tiles (tile
counts uniform across cores -> one SPMD program; the kernel is compiled per
call so all binning is static). The z scatter is a one-hot matmul with
Sfac[e,t] = (iota[t]==t_e)*F[t], F[t]=exp(si[t]/2); the fused accum_out of that
op delivers the per-edge dst attention term si, and F cancels in the z/d
normalization. Per-edge src features come from one dma_gather of 768B rows
[1 | h | sj | pad]. The tail (per-relation q/k/v, psi, channel softmax) is
node-parallel; one AllGather bridges layer1 -> layer2.
"""
import numpy as np

R = 8
N = 50000
E = 800000
NCORES = 8
Nc = N // NCORES            # 6250
TB = 128
RNc = R * Nc                # 50000
RNc_pad = ((RNc + TB - 1) // TB) * TB   # 50048
NBLK = RNc_pad // TB        # 391
SPLITS = [0, 16672, 33344, N]
NG = 3
C1, C2 = 128, 64
ROW1, ROW2 = 192, 128       # gathered row length in f32 words
EPS = 1e-30
WBLK = 8                    # blocks per gather window
NW = (NBLK + WBLK - 1) // WBLK


def _host_prep(edge_index, edge_type):
    src = edge_index[0].astype(np.int64)
    dst = edge_index[1].astype(np.int64)
    et = edge_type.astype(np.int64)
    percore = []
    counts = np.zeros((NCORES, NBLK, NG), np.int64)
    for m in range(NCORES):
        sel = (dst // Nc) == m
        s, d, r = src[sel], dst[sel], et[sel]
        t = (d - m * Nc) * 8 + r
        blk = t // TB
        grp = np.digitize(s, SPLITS[1:NG])
        order = np.lexsort((t, grp, blk))
        s, t, r, blk, grp = s[order], t[order], r[order], blk[order], grp[order]
        percore.append((s, t, r, blk, grp))
        for g in range(NG):
            np.add.at(counts[m, :, g], blk[grp == g], 1)
    tiles = np.maximum(-(-counts.max(0) // 128), 1)   # [NBLK, NG]

    # column maps, shared across cores
    colmap = {}
    call_cols = np.zeros((NW, NG), np.int64)
    for w in range(NW):
        for g in range(NG):
            c = 0
            for b in range(w * WBLK, min((w + 1) * WBLK, NBLK)):
                for k in range(int(tiles[b, g])):
                    colmap[(b, g, k)] = c
                    c += 1
            call_cols[w, g] = c
    call_off = np.zeros((NW, NG), np.int64)
    acc = 0
    for w in range(NW):
        for g in range(NG):
            call_off[w, g] = acc
            acc += call_cols[w, g]
    ncols = int(acc)
    gcols = [int(call_cols[:, g].sum()) for g in range(NG)]
    gcol_off = np.zeros((NW, NG), np.int64)
    for g in range(NG):
        a = 0
        for w in range(NW):
            gcol_off[w, g] = a
            a += call_cols[w, g]

    tabs = []
    for m in range(NCORES):
        s, t, r, blk, grp = percore[m]
        tcol = np.zeros(ncols * 128, np.float32)
        etv = np.zeros(ncols * 128, np.float32)
        mask = np.full(ncols * 128, -1e30, np.float32)
        idxs = [np.zeros(gcols[g] * 128, np.int64) for g in range(NG)]
        for b in range(NBLK):
            w = b // WBLK
            for g in range(NG):
                bm = (blk == b) & (grp == g)
                cnt = int(bm.sum())
                c0 = colmap[(b, g, 0)]
                o = (int(call_off[w, g]) + c0) * 128
                tcol[o:o + cnt] = (t[bm] - b * TB).astype(np.float32)
                etv[o:o + cnt] = r[bm].astype(np.float32)
                mask[o:o + cnt] = 0.0
                io = (int(gcol_off[w, g]) + c0) * 128
                idxs[g][io:io + cnt] = s[bm] - SPLITS[g]

        def wrap16(a):
            ar = a.reshape(-1, 16).T
            return np.ascontiguousarray(np.tile(ar, (8, 1))).astype(np.int16)

        tabs.append(dict(
            tcol=np.ascontiguousarray(tcol.reshape(-1, 128).T),
            etv=np.ascontiguousarray(etv.reshape(-1, 128).T),
            mask=np.ascontiguousarray(mask.reshape(-1, 128).T),
            idx=[wrap16(idxs[g]) for g in range(NG)],
        ))
    meta = dict(tiles=tiles, colmap=colmap, call_cols=call_cols,
                call_off=call_off, gcol_off=gcol_off, ncols=ncols, gcols=gcols)
    return tabs, meta


def _emit(nc, tc, bass, mybir, meta, H):
    import os
    _stop = os.environ.get("BRGCN_STOP", "")
    dt = mybir.dt
    f32 = dt.float32
    A = mybir.AluOpType
    AF = mybir.ActivationFunctionType
    AX = mybir.AxisListType
    tiles = meta['tiles']; colmap = meta['colmap']
    call_cols = meta['call_cols']; call_off = meta['call_off']
    gcol_off = meta['gcol_off']; ncols = meta['ncols']

    with tc.tile_pool(name="persist", bufs=1) as pp:
        iota128 = pp.tile([128, 128], f32, tag="iota128")
        nc.sync.dma_start(iota128[:], H['iota128'][:])
        iota8 = pp.tile([128, 8], f32, tag="iota8")
        nc.sync.dma_start(iota8[:], H['iota8'][:])
        ones1 = pp.tile([1, 128], f32, tag="ones1")
        nc.vector.memset(ones1[:], 1.0)
        ident = pp.tile([128, 128], f32, tag="ident")
        nc.sync.dma_start(ident[:], H['ident'][:])
        sel64 = pp.tile([128, 64], f32, tag="sel64")
        nc.sync.dma_start(sel64[:], H['sel64'][:])
        tcolt = pp.tile([128, ncols], f32, tag="tcolt")
        nc.sync.dma_start(tcolt[:], H['tcol'][:])
        etvt = pp.tile([128, ncols], f32, tag="etvt")
        nc.sync.dma_start(etvt[:], H['etv'][:])
        maskt = pp.tile([128, ncols], f32, tag="maskt")
        nc.sync.dma_start(maskt[:], H['mask'][:])
        siF = pp.tile([128, ncols], f32, tag="siF")
        warr = pp.tile([128, ncols], f32, tag="warr")
        hT1 = pp.tile([C1, Nc], f32, tag="hT1")
        hT2 = pp.tile([C2, Nc], f32, tag="hT2")

        for lay in (1, 2):
            C = C1 if lay == 1 else C2
            ROWW = ROW1 if lay == 1 else ROW2
            hT = hT1 if lay == 1 else hT2
            hx = H['hx1'] if lay == 1 else H['hx2']
            zT = H['zT1'] if lay == 1 else H['zT2']
            Fd = H['F1'] if lay == 1 else H['F2']
            Wcat = H['Wcat1'] if lay == 1 else H['Wcat2']
            attiT = H['attiT1'] if lay == 1 else H['attiT2']
            Wq = H['Wq1'] if lay == 1 else H['Wq2']
            Wk = H['Wk1'] if lay == 1 else H['Wk2']
            Wv = H['Wv1'] if lay == 1 else H['Wv2']
            Ws = H['Ws1'] if lay == 1 else H['Ws2']
            CIN = 128

            # source tiles for phase A (full N in <=128-node pieces)
            if lay == 1:
                abatches = []
                for k0 in range(0, N, 512):
                    tot = min(512, N - k0)
                    abatches.append((H['xT'][:, k0:k0 + tot], k0, tot))
                xt_loc = H['xTloc']
            else:
                abatches = []
                for g in range(NCORES):
                    for k0 in range(0, Nc, 512):
                        tot = min(512, Nc - k0)
                        abatches.append(
                            (H['x2T'][g * 128:(g + 1) * 128, k0:k0 + tot],
                             g * Nc + k0, tot))
                xt_loc = H['o1T'][:, :]

            # ---------------- Phase A ----------------
            with tc.tile_pool(name="pa", bufs=3) as pa, \
                 tc.tile_pool(name="pap", bufs=2, space="PSUM") as pap:
                wcat = pa.tile([CIN, C + 8], f32, tag="wcat")
                nc.sync.dma_start(wcat[:], Wcat[:])
                for (src_ap, n0, tot) in abatches:
                    nt = (tot + 127) // 128
                    xt_sb = pa.tile([CIN, 512], f32, tag="xt")
                    nc.sync.dma_start(xt_sb[:, :tot], src_ap)
                    stg = pa.tile([128, 4 * (1 + C + 8)], f32, tag="stg")
                    W = 1 + C + 8
                    for j in range(nt):
                        nn = min(128, tot - j * 128)
                        hps = pap.tile([128, C + 8], f32, tag="hps")
                        nc.tensor.matmul(hps[:nn, :],
                                         xt_sb[:, j * 128:j * 128 + nn],
                                         wcat[:], start=True, stop=True)
                        nc.vector.memset(stg[:nn, j * W:j * W + 1], 1.0)
                        nc.scalar.activation(stg[:nn, j * W + 1:(j + 1) * W],
                                             hps[:nn, :], AF.Copy)
                    nc.sync.dma_start(
                        bass.AP(hx.tensor, n0 * ROWW,
                                [[ROWW, 128], [ROWW * 128, nt], [1, W]]),
                        stg[:, 0:nt * W].rearrange("p (j w) -> p j w", w=W))
                # hT_local
                for k in range((Nc + 127) // 128):
                    n0 = k * 128
                    nn = min(128, Nc - n0)
                    xt_sb = pa.tile([CIN, 128], f32, tag="xt")
                    nc.sync.dma_start(xt_sb[:, :nn], xt_loc[:, n0:n0 + nn])
                    hps = pap.tile([128, 128], f32, tag="hps2")
                    nc.tensor.matmul(hps[:C, :nn], wcat[:, 0:C], xt_sb[:, :nn],
                                     start=True, stop=True)
                    nc.scalar.activation(hT[:C, n0:n0 + nn], hps[:C, :nn],
                                         AF.Copy)
                # F table
                atti = pa.tile([C, 8], f32, tag="atti")
                nc.sync.dma_start(atti[:], attiT[:])
                FT = pa.tile([8, Nc], f32, tag="FT")
                for k in range((Nc + 127) // 128):
                    n0 = k * 128
                    nn = min(128, Nc - n0)
                    sps = pap.tile([8, 128], f32, tag="sps")
                    nc.tensor.matmul(sps[:8, :nn], atti[:C, :],
                                     hT[:C, n0:n0 + nn], start=True, stop=True)
                    nc.scalar.activation(FT[:, n0:n0 + nn], sps[:8, :nn],
                                         AF.Exp, scale=0.5)
                nc.sync.dma_start(bass.AP(Fd.tensor, 0, [[1, 8], [8, Nc]]),
                                  FT[:, :Nc])
                nc.sync.dma_start(Fd[RNc:RNc_pad],
                                  ones1[0:1, 0:RNc_pad - RNc])

            if _stop == f"A{lay}":
                return
            # ---------------- Phase B: edges ----------------
            with tc.tile_pool(name="pb", bufs=2) as pb, \
                 tc.tile_pool(name="pbs", bufs=4) as pbs, \
                 tc.tile_pool(name="sfp", bufs=4 * WBLK + 8) as sfp, \
                 tc.tile_pool(name="pbp", bufs=2, space="PSUM") as pbp, \
                 tc.tile_pool(name="zpp", bufs=3, space="PSUM") as zpp:
                for w in range(NW):
                    b0 = w * WBLK
                    b1 = min(b0 + WBLK, NBLK)
                    stgs = []
                    for g in range(NG):
                        cols = int(call_cols[w, g])
                        gt = pb.tile([128, cols * ROWW], f32, tag=f"G{g}")
                        io = int(gcol_off[w, g])
                        idxt = pbs.tile([128, cols * 8], dt.int16, tag=f"ix{g}")
                        nc.sync.dma_start(
                            idxt[:], H['idx'][g][:, io * 8:(io + cols) * 8])
                        nc.gpsimd.dma_gather(
                            gt[:].rearrange("p (c e) -> p c e", e=ROWW),
                            hx[SPLITS[g]:SPLITS[g + 1], 0:ROWW],
                            idxt[:], cols * 128, cols * 128, ROWW,
                            elem_step=ROWW)
                        stgs.append(gt)
                    fsb = pbs.tile([1, WBLK * TB], f32, tag="fsb")
                    nc.sync.dma_start(fsb[0:1, 0:(b1 - b0) * TB],
                                      Fd[b0 * TB:b1 * TB])
                    sfacs = {}
                    for b in range(b0, b1):
                        frep = pbp.tile([128, TB], f32, tag="frep")
                        nc.tensor.matmul(
                            frep[:], ones1[:],
                            fsb[0:1, (b - b0) * TB:(b - b0 + 1) * TB],
                            start=True, stop=True)
                        for g in range(NG):
                            for k in range(int(tiles[b, g])):
                                gc = int(call_off[w, g]) + colmap[(b, g, k)]
                                sf = sfp.tile([128, TB], dt.bfloat16,
                                              tag="sfac")
                                nc.vector.scalar_tensor_tensor(
                                    sf[:], iota128[:], tcolt[:, gc:gc + 1],
                                    frep[:], A.is_equal, A.mult,
                                    accum_out=siF[:, gc:gc + 1])
                                sfacs[(b, g, k)] = sf
                    # alpha / w for the whole window
                    gc0 = int(call_off[w, 0])
                    gc1 = int(call_off[w, NG - 1] + call_cols[w, NG - 1])
                    cw = gc1 - gc0
                    sjsel = pbs.tile([128, cw], f32, tag="sjsel")
                    for g in range(NG):
                        cols = int(call_cols[w, g])
                        o = int(call_off[w, g]) - gc0
                        G3 = stgs[g][:].rearrange("p (c e) -> p c e", e=ROWW)
                        s8 = pbs.tile([128, cols * 8], f32, tag="s8")
                        s83 = s8[:].rearrange("p (c e) -> p c e", e=8)
                        nc.vector.tensor_tensor(
                            s83,
                            iota8[:].unsqueeze(1).to_broadcast((128, cols, 8)),
                            etvt[:, gc0 + o:gc0 + o + cols]
                                .unsqueeze(2).to_broadcast((128, cols, 8)),
                            A.is_equal)
                        nc.vector.tensor_tensor(
                            s83, s83, G3[:, :, 1 + C:1 + C + 8], A.mult)
                        nc.vector.tensor_reduce(
                            sjsel[:, o:o + cols], s83, AX.X, A.add)
                    lnf = pbs.tile([128, cw], f32, tag="lnf")
                    nc.scalar.activation(lnf[:], siF[:, gc0:gc1], AF.Ln)
                    calp = pbs.tile([128, cw], f32, tag="calp")
                    nc.vector.scalar_tensor_tensor(
                        calp[:], lnf[:], 2.0, sjsel[:], A.mult, A.add)
                    nc.vector.scalar_tensor_tensor(
                        calp[:], calp[:], 0.2, calp[:], A.mult, A.max)
                    nc.vector.scalar_tensor_tensor(
                        calp[:], calp[:], 80.0, maskt[:, gc0:gc1],
                        A.min, A.add)
                    nc.scalar.activation(warr[:, gc0:gc1], calp[:], AF.Exp)
                    # weighting + z matmuls + per-block epilogue
                    for b in range(b0, b1):
                        zps = zpp.tile([128, 1 + C], f32, tag="zps")
                        first = True
                        nt = [(g, k) for g in range(NG)
                              for k in range(int(tiles[b, g]))]
                        for (g, k) in nt:
                            gc = int(call_off[w, g]) + colmap[(b, g, k)]
                            c = colmap[(b, g, k)]
                            G3 = stgs[g][:].rearrange("p (c e) -> p c e",
                                                      e=ROWW)
                            whj = pbs.tile([128, 1 + C], dt.bfloat16,
                                           tag="whj")
                            if gc % 2 == 0:
                                nc.scalar.activation(
                                    whj[:], G3[:, c, 0:1 + C], AF.Copy,
                                    scale=warr[:, gc:gc + 1])
                            else:
                                nc.vector.tensor_scalar_mul(
                                    whj[:], G3[:, c, 0:1 + C],
                                    warr[:, gc:gc + 1])
                            nc.tensor.matmul(zps[:], sfacs[(b, g, k)][:],
                                             whj[:], start=first,
                                             stop=((g, k) == nt[-1]))
                            first = False
                        rec = pbs.tile([128, 1], f32, tag="rec")
                        nc.vector.tensor_scalar(rec[:], zps[:, 0:1], EPS,
                                                None, A.add)
                        nc.vector.reciprocal(rec[:], rec[:])
                        zn = pbs.tile([128, C], f32, tag="zn")
                        nc.vector.tensor_scalar_mul(zn[:], zps[:, 1:1 + C],
                                                    rec[:])
                        ztp = pbp.tile([C, 128], f32, tag="ztp")
                        nc.tensor.transpose(ztp[:], zn[:], ident[:])
                        if b % 4 == 0:
                            zt4 = pb.tile([C, 4 * TB], f32, tag="zt4")
                        nc.scalar.activation(
                            zt4[:C, (b % 4) * TB:(b % 4 + 1) * TB], ztp[:],
                            AF.Copy)
                        if b % 4 == 3 or b == NBLK - 1:
                            bb0 = (b // 4) * 4
                            nc.sync.dma_start(
                                zT[:C, bb0 * TB:(b + 1) * TB],
                                zt4[:C, 0:(b + 1 - bb0) * TB])

            if _stop == f"B{lay}":
                return
            # ---------------- Phase C: tail ----------------
            with tc.tile_pool(name="pc", bufs=3) as pc, \
                 tc.tile_pool(name="pcw", bufs=1) as pcw, \
                 tc.tile_pool(name="pcp", bufs=1, space="PSUM") as pcp, \
                 tc.tile_pool(name="pcq", bufs=1, space="PSUM") as pcq:
                wq = pcw.tile([C, 8 * C], f32, tag="wq")
                nc.sync.dma_start(wq[:], Wq[:])
                wk = pcw.tile([C, 8 * C], f32, tag="wk")
                nc.sync.dma_start(wk[:], Wk[:])
                wv = pcw.tile([C, 8 * C], f32, tag="wv")
                nc.sync.dma_start(wv[:], Wv[:])
                ws = pcw.tile([C, C], f32, tag="ws")
                nc.sync.dma_start(ws[:], Ws[:])
                for k in range((Nc + 127) // 128):
                    n0 = k * 128
                    nn = min(128, Nc - n0)
                    ztc = pc.tile([C, 1024], f32, tag="ztc")
                    nc.sync.dma_start(ztc[:C, 0:nn * 8],
                                      zT[:C, n0 * 8:(n0 + nn) * 8])
                    zt3 = ztc[:C, 0:nn * 8].rearrange("c (n r) -> c r n", r=8)
                    kps = pcp.tile([C, 128], f32, tag="kps")
                    vps = pcp.tile([C, 128], f32, tag="vps")
                    for r in range(8):
                        nc.tensor.matmul(kps[:C, :nn],
                                         wk[:, r * C:(r + 1) * C],
                                         zt3[:, r, :], start=(r == 0),
                                         stop=(r == 7))
                    for r in range(8):
                        nc.tensor.matmul(vps[:C, :nn],
                                         wv[:, r * C:(r + 1) * C],
                                         zt3[:, r, :], start=(r == 0),
                                         stop=(r == 7))
                    ks = pc.tile([C, 128], f32, tag="ks")
                    nc.scalar.activation(ks[:C, :nn], kps[:C, :nn], AF.Copy)
                    pps = pcp.tile([8, 128], f32, tag="pps")
                    for r in range(8):
                        qps = pcq.tile([C, 128], f32, tag="qps")
                        nc.tensor.matmul(qps[:C, :nn],
                                         wq[:, r * C:(r + 1) * C],
                                         zt3[:, r, :], start=True, stop=True)
                        tmp = pc.tile([C, 128], f32, tag="tmp")
                        nc.vector.tensor_tensor(tmp[:C, :nn], qps[:C, :nn],
                                                ks[:C, :nn], A.mult)
                        nc.tensor.matmul(pps[:8, :nn],
                                         sel64[:C, r * 8:(r + 1) * 8],
                                         tmp[:C, :nn], start=(r == 0),
                                         stop=(r == 7))
                    psis = pc.tile([8, 128], f32, tag="psis")
                    nc.scalar.activation(psis[:8, :nn], pps[:8, :nn], AF.Copy)
                    ptp = pcq.tile([128, 8], f32, tag="ptp")
                    nc.tensor.transpose(ptp[:nn, :], psis[:8, :nn],
                                        ident[0:8, 0:8])
                    psiT = pc.tile([128, 8], f32, tag="psiT")
                    nc.scalar.activation(psiT[:nn, :], ptp[:nn, :], AF.Copy)
                    vs = pc.tile([C, 128], f32, tag="vs")
                    nc.scalar.activation(vs[:C, :nn], vps[:C, :nn], AF.Copy)
                    vtp = pcq.tile([128, C], f32, tag="vtp")
                    nc.tensor.transpose(vtp[:nn, :], vs[:C, :nn],
                                        ident[0:C, 0:C])
                    vsum = pc.tile([128, C], f32, tag="vsum")
                    nc.scalar.activation(vsum[:nn, :], vtp[:nn, :], AF.Copy)
                    bps = pcq.tile([128, C], f32, tag="bps")
                    nc.tensor.matmul(bps[:nn, :], hT[:C, n0:n0 + nn], ws[:],
                                     start=True, stop=True)
                    # delta
                    g2 = pc.tile([128, 8 * C], f32, tag="g2")
                    g23 = g2[:nn, :].rearrange("p (r c) -> p r c", r=8)
                    nc.vector.tensor_tensor(
                        g23,
                        vsum[:nn, :].unsqueeze(1).to_broadcast((nn, 8, C)),
                        psiT[:nn, :].unsqueeze(2).to_broadcast((nn, 8, C)),
                        A.mult)
                    nc.vector.tensor_tensor(
                        g23, g23,
                        bps[:nn, :].unsqueeze(1).to_broadcast((nn, 8, C)),
                        A.add)
                    m8 = pc.tile([128, 8], f32, tag="m8")
                    nc.vector.tensor_reduce(m8[:nn, :], g23, AX.X, A.max)
                    nc.vector.tensor_scalar_mul(m8[:nn, :], m8[:nn, :], -1.0)
                    ssum = pc.tile([128, 8], f32, tag="ssum")
                    for r in range(8):
                        nc.scalar.activation(
                            g2[:nn, r * C:(r + 1) * C],
                            g2[:nn, r * C:(r + 1) * C], AF.Exp,
                            bias=m8[:nn, r:r + 1],
                            accum_out=ssum[:nn, r:r + 1])
                    nc.vector.reciprocal(ssum[:nn, :], ssum[:nn, :])
                    for r in range(8):
                        nc.scalar.activation(
                            g2[:nn, r * C:(r + 1) * C],
                            g2[:nn, r * C:(r + 1) * C], AF.Copy,
                            scale=ssum[:nn, r:r + 1])
                    outc = pc.tile([128, C], f32, tag="outc")
                    nc.vector.tensor_reduce(
                        outc[:nn, :],
                        g2[:nn, :].rearrange("p (r c) -> p c r", r=8),
                        AX.X, A.add)
                    if lay == 1:
                        otp = pcq.tile([C, 128], f32, tag="vtp")
                        nc.tensor.transpose(otp[:C, :nn], outc[:nn, :],
                                            ident[0:nn, 0:nn])
                        ot = pc.tile([C, 128], f32, tag="ot")
                        nc.scalar.activation(ot[:C, :nn], otp[:C, :nn],
                                             AF.Copy)
                        nc.sync.dma_start(H['o1T'][:, n0:n0 + nn],
                                          ot[:C, :nn])
                    else:
                        m1 = pc.tile([128, 1], f32, tag="m1")
                        nc.vector.tensor_reduce(m1[:nn, :], outc[:nn, :],
                                                AX.X, A.max)
                        nc.vector.tensor_scalar_mul(m1[:nn, :], m1[:nn, :],
                                                    -1.0)
                        sc = pc.tile([128, C], f32, tag="sc")
                        s1 = pc.tile([128, 1], f32, tag="s1")
                        nc.scalar.activation(sc[:nn, :], outc[:nn, :], AF.Exp,
                                             bias=m1[:nn, :],
                                             accum_out=s1[:nn, :])
                        lns = pc.tile([128, 1], f32, tag="lns")
                        nc.scalar.activation(lns[:nn, :], s1[:nn, :], AF.Ln)
                        res = pc.tile([128, C], f32, tag="res")
                        nc.vector.scalar_tensor_tensor(
                            res[:nn, :], outc[:nn, :], m1[:nn, :],
                            lns[:nn, :].to_broadcast((nn, C)),
                            A.add, A.subtract)
                        nc.sync.dma_start(H['out'][n0:n0 + nn, :],
                                          res[:nn, :])
            if _stop == f"C{lay}":
                return
            if lay == 1:
                nc.gpsimd.collective_compute(
                    "AllGather", A.bypass,
                    replica_groups=[list(range(NCORES))],
                    ins=[H['o1T'][:]],
                    outs=[H['x2T'][:]])


def kernel(**inputs):
    import concourse.bass as bass
    import concourse.bacc as bacc
    import concourse.mybir as mybir
    import concourse.tile as tile
    from concourse.bass_utils import run_bass_kernel_spmd

    ins = {k: np.asarray(v) for k, v in inputs.items()}
    tabs, meta = _host_prep(ins['edge_index'], ins['edge_type'])
    ncols = meta['ncols']
    gcols = meta['gcols']

    f32 = mybir.dt.float32
    i16 = mybir.dt.int16
    nc = bacc.Bacc("TRN2", target_bir_lowering=False, debug=False,
                   num_devices=NCORES)

    def din(name, shape, dtype=f32):
        return nc.dram_tensor(name, list(shape), dtype,
                              kind="ExternalInput").ap()

    H = {}
    H['xT'] = din("xT", [128, N])
    H['xTloc'] = din("xTloc", [128, Nc])
    H['iota128'] = din("iota128", [128, 128])
    H['iota8'] = din("iota8", [128, 8])
    H['ident'] = din("ident", [128, 128])
    H['sel64'] = din("sel64", [128, 64])
    H['tcol'] = din("tcol", [128, ncols])
    H['etv'] = din("etv", [128, ncols])
    H['mask'] = din("mask", [128, ncols])
    H['idx'] = [din(f"idx{g}", [128, gcols[g] * 8], i16) for g in range(NG)]
    for l, c in ((1, C1), (2, C2)):
        H[f'Wcat{l}'] = din(f"Wcat{l}", [128, c + 8])
        H[f'attiT{l}'] = din(f"attiT{l}", [c, 8])
        H[f'Wq{l}'] = din(f"Wq{l}", [c, 8 * c])
        H[f'Wk{l}'] = din(f"Wk{l}", [c, 8 * c])
        H[f'Wv{l}'] = din(f"Wv{l}", [c, 8 * c])
        H[f'Ws{l}'] = din(f"Ws{l}", [c, c])
    H['hx1'] = nc.dram_tensor("hx1", [N, ROW1], f32).ap()
    H['hx2'] = nc.dram_tensor("hx2", [N, ROW2], f32).ap()
    H['zT1'] = nc.dram_tensor("zT1", [C1, RNc_pad], f32).ap()
    H['zT2'] = nc.dram_tensor("zT2", [C2, RNc_pad], f32).ap()
    H['F1'] = nc.dram_tensor("F1", [RNc_pad], f32).ap()
    H['F2'] = nc.dram_tensor("F2", [RNc_pad], f32).ap()
    H['o1T'] = nc.dram_tensor("o1T", [128, Nc], f32).ap()
    H['x2T'] = nc.dram_tensor("x2T", [NCORES * 128, Nc], f32,
                              addr_space="Shared").ap()
    H['out'] = nc.dram_tensor("out", [Nc, C2], f32,
                              kind="ExternalOutput").ap()

    with tile.TileContext(nc) as tc:
        _emit(nc, tc, bass, mybir, meta, H)
    nc.compile()

    # host-side constant inputs
    x = ins['x'].astype(np.float32)
    iota128 = np.broadcast_to(np.arange(128, dtype=np.float32), (128, 128))
    iota8 = np.broadcast_to(np.arange(8, dtype=np.float32), (128, 8))
    ident = np.eye(128, dtype=np.float32)
    sel64 = np.zeros((128, 64), np.float32)
    for r in range(8):
        sel64[:, r * 8 + r] = 1.0

    common = dict(
        xT=np.ascontiguousarray(x.T),
        iota128=np.ascontiguousarray(iota128),
        iota8=np.ascontiguousarray(iota8),
        ident=ident, sel64=sel64,
    )
    for l, c in ((1, C1), (2, C2)):
        att = ins[f'att{l}'].astype(np.float32)
        Wn = ins[f'Wn{l}'].astype(np.float32)
        common[f'Wcat{l}'] = np.ascontiguousarray(
            np.concatenate([Wn, Wn @ att[:, c:].T], axis=1))
        common[f'attiT{l}'] = np.ascontiguousarray(att[:, :c].T)
        for nm in ('Wq', 'Wk', 'Wv'):
            W = ins[f'{nm}{l}'].astype(np.float32)
            common[f'{nm}{l}'] = np.ascontiguousarray(
                W.transpose(1, 0, 2).reshape(c, 8 * c))
        common[f'Ws{l}'] = ins[f'Ws{l}'].astype(np.float32)

    in_maps = []
    for m in range(NCORES):
        im = dict(common)
        im['xTloc'] = np.ascontiguousarray(x.T[:, m * Nc:(m + 1) * Nc])
        im['tcol'] = tabs[m]['tcol']
        im['etv'] = tabs[m]['etv']
        im['mask'] = tabs[m]['mask']
        for g in range(NG):
            im[f'idx{g}'] = tabs[m]['idx'][g]
        in_maps.append(im)

    global _LAST_BUILD, LAST_EXEC_NS
    _LAST_BUILD = (nc, in_maps)
    res = run_bass_kernel_spmd(nc, in_maps, list(range(NCORES)))
    LAST_EXEC_NS = res.exec_time_ns
    out = np.concatenate([res.results[m]['out'] for m in range(NCORES)], 0)
    return out.astype(np.float32)


LAST_EXEC_NS = None
_LAST_BUILD = None


# revision 10
# speedup vs baseline: 5.0340x; 5.0340x over previous
"""BRGCN (2-layer relational GCN with bilinear attention) on 8 TRN2 NeuronCores.

Sharding: edges are sharded by dst node (6250 nodes/core) so the
(relation, dst) segment softmax and the z scatter are fully core-local; node
features are replicated. Per core the edges are sorted by t = dst_local*8+rel,
binned into 391 blocks of 128 t-rows and padded into 128-edge K-tiles (tile
counts uniform across cores -> one SPMD program; the kernel is compiled per
call so all binning is static). The z scatter is a one-hot matmul with
Sfac[e,t] = (iota[t]==t_e)*F[t], F[t]=exp(si[t]/2); the fused accum_out of that
op delivers the per-edge dst attention term si, and F cancels in the z/d
normalization. Per-edge src features come from one dma_gather of 768B rows
[1 | h | sj | pad]. The tail (per-relation q/k/v, psi, channel softmax) is
node-parallel; one AllGather bridges layer1 -> layer2.
"""
import numpy as np

R = 8
N = 50000
E = 800000
NCORES = 8
Nc = N // NCORES            # 6250
TB = 128
RNc = R * Nc                # 50000
RNc_pad = ((RNc + TB - 1) // TB) * TB   # 50048
NBLK = RNc_pad // TB        # 391
SPLITS = [0, 16672, 33344, N]
NG = 3
C1, C2 = 128, 64
ROW1, ROW2 = 192, 128       # gathered row length in f32 words
EPS = 1e-30
WBLK = 8                    # blocks per gather window
NW = (NBLK + WBLK - 1) // WBLK


def _host_prep(edge_index, edge_type):
    src = edge_index[0].astype(np.int64)
    dst = edge_index[1].astype(np.int64)
    et = edge_type.astype(np.int64)
    percore = []
    counts = np.zeros((NCORES, NBLK, NG), np.int64)
    for m in range(NCORES):
        sel = (dst // Nc) == m
        s, d, r = src[sel], dst[sel], et[sel]
        t = (d - m * Nc) * 8 + r
        blk = t // TB
        grp = np.digitize(s, SPLITS[1:NG])
        order = np.lexsort((t, grp, blk))
        s, t, r, blk, grp = s[order], t[order], r[order], blk[order], grp[order]
        percore.append((s, t, r, blk, grp))
        for g in range(NG):
            np.add.at(counts[m, :, g], blk[grp == g], 1)
    tiles = np.maximum(-(-counts.max(0) // 128), 1)   # [NBLK, NG]

    # column maps, shared across cores
    colmap = {}
    call_cols = np.zeros((NW, NG), np.int64)
    for w in range(NW):
        for g in range(NG):
            c = 0
            for b in range(w * WBLK, min((w + 1) * WBLK, NBLK)):
                for k in range(int(tiles[b, g])):
                    colmap[(b, g, k)] = c
                    c += 1
            call_cols[w, g] = c
    call_off = np.zeros((NW, NG), np.int64)
    acc = 0
    for w in range(NW):
        for g in range(NG):
            call_off[w, g] = acc
            acc += call_cols[w, g]
    ncols = int(acc)
    gcols = [int(call_cols[:, g].sum()) for g in range(NG)]
    gcol_off = np.zeros((NW, NG), np.int64)
    for g in range(NG):
        a = 0
        for w in range(NW):
            gcol_off[w, g] = a
            a += call_cols[w, g]

    tabs = []
    for m in range(NCORES):
        s, t, r, blk, grp = percore[m]
        tcol = np.zeros(ncols * 128, np.float32)
        etv = np.zeros(ncols * 128, np.float32)
        mask = np.full(ncols * 128, -1e30, np.float32)
        idxs = [np.zeros(gcols[g] * 128, np.int64) for g in range(NG)]
        for b in range(NBLK):
            w = b // WBLK
            for g in range(NG):
                bm = (blk == b) & (grp == g)
                cnt = int(bm.sum())
                c0 = colmap[(b, g, 0)]
                o = (int(call_off[w, g]) + c0) * 128
                tcol[o:o + cnt] = (t[bm] - b * TB).astype(np.float32)
                etv[o:o + cnt] = r[bm].astype(np.float32)
                mask[o:o + cnt] = 0.0
                io = (int(gcol_off[w, g]) + c0) * 128
                idxs[g][io:io + cnt] = s[bm] - SPLITS[g]

        def wrap16(a):
            ar = a.reshape(-1, 16).T
            return np.ascontiguousarray(np.tile(ar, (8, 1))).astype(np.int16)

        tabs.append(dict(
            tcol=np.ascontiguousarray(tcol.reshape(-1, 128).T),
            etv=np.ascontiguousarray(etv.reshape(-1, 128).T),
            mask=np.ascontiguousarray(mask.reshape(-1, 128).T),
            idx=[wrap16(idxs[g]) for g in range(NG)],
        ))
    meta = dict(tiles=tiles, colmap=colmap, call_cols=call_cols,
                call_off=call_off, gcol_off=gcol_off, ncols=ncols, gcols=gcols)
    return tabs, meta


def _emit(nc, tc, bass, mybir, meta, H):
    import os
    _stop = os.environ.get("BRGCN_STOP", "")
    dt = mybir.dt
    f32 = dt.float32
    A = mybir.AluOpType
    AF = mybir.ActivationFunctionType
    AX = mybir.AxisListType
    tiles = meta['tiles']; colmap = meta['colmap']
    call_cols = meta['call_cols']; call_off = meta['call_off']
    gcol_off = meta['gcol_off']; ncols = meta['ncols']

    with tc.tile_pool(name="persist", bufs=1) as pp:
        iota128 = pp.tile([128, 128], f32, tag="iota128")
        nc.sync.dma_start(iota128[:], H['iota128'][:])
        iota8 = pp.tile([128, 8], f32, tag="iota8")
        nc.sync.dma_start(iota8[:], H['iota8'][:])
        ones1 = pp.tile([1, 128], f32, tag="ones1")
        nc.vector.memset(ones1[:], 1.0)
        ident = pp.tile([128, 128], f32, tag="ident")
        nc.sync.dma_start(ident[:], H['ident'][:])
        sel64 = pp.tile([128, 64], f32, tag="sel64")
        nc.sync.dma_start(sel64[:], H['sel64'][:])
        tcolt = pp.tile([128, ncols], f32, tag="tcolt")
        nc.sync.dma_start(tcolt[:], H['tcol'][:])
        etvt = pp.tile([128, ncols], f32, tag="etvt")
        nc.sync.dma_start(etvt[:], H['etv'][:])
        maskt = pp.tile([128, ncols], f32, tag="maskt")
        nc.sync.dma_start(maskt[:], H['mask'][:])
        siF = pp.tile([128, ncols], f32, tag="siF")
        warr = pp.tile([128, ncols], f32, tag="warr")
        hT1 = pp.tile([C1, Nc], f32, tag="hT1")
        hT2 = pp.tile([C2, Nc], f32, tag="hT2")

        for lay in (1, 2):
            C = C1 if lay == 1 else C2
            ROWW = ROW1 if lay == 1 else ROW2
            hT = hT1 if lay == 1 else hT2
            hx = H['hx1'] if lay == 1 else H['hx2']
            zT = H['zT1'] if lay == 1 else H['zT2']
            Fd = H['F1'] if lay == 1 else H['F2']
            Wcat = H['Wcat1'] if lay == 1 else H['Wcat2']
            attiT = H['attiT1'] if lay == 1 else H['attiT2']
            Wq = H['Wq1'] if lay == 1 else H['Wq2']
            Wk = H['Wk1'] if lay == 1 else H['Wk2']
            Wv = H['Wv1'] if lay == 1 else H['Wv2']
            Ws = H['Ws1'] if lay == 1 else H['Ws2']
            CIN = 128

            # source tiles for phase A (full N in <=128-node pieces)
            if lay == 1:
                atiles = [(H['xT'][:, k * 128:k * 128 + min(128, N - k * 128)],
                           k * 128, min(128, N - k * 128))
                          for k in range((N + 127) // 128)]
                xt_loc = H['xT'][:, 0:Nc]  # placeholder, replaced below
                xt_loc = H['xTloc']
            else:
                atiles = []
                for g in range(NCORES):
                    for k in range((Nc + 127) // 128):
                        nn = min(128, Nc - k * 128)
                        atiles.append((H['x2T'][g * 128:(g + 1) * 128,
                                                k * 128:k * 128 + nn],
                                       g * Nc + k * 128, nn))
                xt_loc = H['o1T'][:, :]

            # ---------------- Phase A ----------------
            with tc.tile_pool(name="pa", bufs=3) as pa, \
                 tc.tile_pool(name="pap", bufs=2, space="PSUM") as pap:
                wcat = pa.tile([CIN, C + 8], f32, tag="wcat")
                nc.sync.dma_start(wcat[:], Wcat[:])
                for (src_ap, n0, nn) in atiles:
                    xt_sb = pa.tile([CIN, 128], f32, tag="xt")
                    nc.sync.dma_start(xt_sb[:, :nn], src_ap)
                    hps = pap.tile([128, C + 8], f32, tag="hps")
                    nc.tensor.matmul(hps[:nn, :], xt_sb[:, :nn], wcat[:],
                                     start=True, stop=True)
                    stg = pa.tile([128, 1 + C + 8], f32, tag="stg")
                    nc.vector.memset(stg[:nn, 0:1], 1.0)
                    nc.scalar.activation(stg[:nn, 1:1 + C + 8], hps[:nn, :],
                                         AF.Copy)
                    nc.sync.dma_start(hx[n0:n0 + nn, 0:1 + C + 8], stg[:nn, :])
                # hT_local
                for k in range((Nc + 127) // 128):
                    n0 = k * 128
                    nn = min(128, Nc - n0)
                    xt_sb = pa.tile([CIN, 128], f32, tag="xt")
                    nc.sync.dma_start(xt_sb[:, :nn], xt_loc[:, n0:n0 + nn])
                    hps = pap.tile([128, 128], f32, tag="hps2")
                    nc.tensor.matmul(hps[:C, :nn], wcat[:, 0:C], xt_sb[:, :nn],
                                     start=True, stop=True)
                    nc.scalar.activation(hT[:C, n0:n0 + nn], hps[:C, :nn],
                                         AF.Copy)
                # F table
                atti = pa.tile([C, 8], f32, tag="atti")
                nc.sync.dma_start(atti[:], attiT[:])
                FT = pa.tile([8, Nc], f32, tag="FT")
                for k in range((Nc + 127) // 128):
                    n0 = k * 128
                    nn = min(128, Nc - n0)
                    sps = pap.tile([8, 128], f32, tag="sps")
                    nc.tensor.matmul(sps[:8, :nn], atti[:C, :],
                                     hT[:C, n0:n0 + nn], start=True, stop=True)
                    nc.scalar.activation(FT[:, n0:n0 + nn], sps[:8, :nn],
                                         AF.Exp, scale=0.5)
                nc.sync.dma_start(bass.AP(Fd.tensor, 0, [[1, 8], [8, Nc]]),
                                  FT[:, :Nc])
                nc.sync.dma_start(Fd[RNc:RNc_pad],
                                  ones1[0:1, 0:RNc_pad - RNc])

            if _stop == f"A{lay}":
                return
            # ---------------- Phase B: edges ----------------
            with tc.tile_pool(name="pb", bufs=2) as pb, \
                 tc.tile_pool(name="pbs", bufs=4) as pbs, \
                 tc.tile_pool(name="sfp", bufs=4 * WBLK + 8) as sfp, \
                 tc.tile_pool(name="pbp", bufs=2, space="PSUM") as pbp, \
                 tc.tile_pool(name="zpp", bufs=3, space="PSUM") as zpp:
                for w in range(NW):
                    b0 = w * WBLK
                    b1 = min(b0 + WBLK, NBLK)
                    stgs = []
                    for g in range(NG):
                        cols = int(call_cols[w, g])
                        gt = pb.tile([128, cols * ROWW], f32, tag=f"G{g}")
                        io = int(gcol_off[w, g])
                        idxt = pbs.tile([128, cols * 8], dt.int16, tag=f"ix{g}")
                        nc.sync.dma_start(
                            idxt[:], H['idx'][g][:, io * 8:(io + cols) * 8])
                        nc.gpsimd.dma_gather(
                            gt[:].rearrange("p (c e) -> p c e", e=ROWW),
                            hx[SPLITS[g]:SPLITS[g + 1], 0:ROWW],
                            idxt[:], cols * 128, cols * 128, ROWW,
                            elem_step=ROWW)
                        stgs.append(gt)
                    fsb = pbs.tile([1, WBLK * TB], f32, tag="fsb")
                    nc.sync.dma_start(fsb[0:1, 0:(b1 - b0) * TB],
                                      Fd[b0 * TB:b1 * TB])
                    sfacs = {}
                    for b in range(b0, b1):
                        frep = pbp.tile([128, TB], f32, tag="frep")
                        nc.tensor.matmul(
                            frep[:], ones1[:],
                            fsb[0:1, (b - b0) * TB:(b - b0 + 1) * TB],
                            start=True, stop=True)
                        for g in range(NG):
                            for k in range(int(tiles[b, g])):
                                gc = int(call_off[w, g]) + colmap[(b, g, k)]
                                sf = sfp.tile([128, TB], dt.bfloat16,
                                              tag="sfac")
                                nc.vector.scalar_tensor_tensor(
                                    sf[:], iota128[:], tcolt[:, gc:gc + 1],
                                    frep[:], A.is_equal, A.mult,
                                    accum_out=siF[:, gc:gc + 1])
                                sfacs[(b, g, k)] = sf
                    # alpha / w for the whole window
                    gc0 = int(call_off[w, 0])
                    gc1 = int(call_off[w, NG - 1] + call_cols[w, NG - 1])
                    cw = gc1 - gc0
                    sjsel = pbs.tile([128, cw], f32, tag="sjsel")
                    for g in range(NG):
                        cols = int(call_cols[w, g])
                        o = int(call_off[w, g]) - gc0
                        G3 = stgs[g][:].rearrange("p (c e) -> p c e", e=ROWW)
                        s8 = pbs.tile([128, cols * 8], f32, tag="s8")
                        s83 = s8[:].rearrange("p (c e) -> p c e", e=8)
                        nc.vector.tensor_tensor(
                            s83,
                            iota8[:].unsqueeze(1).to_broadcast((128, cols, 8)),
                            etvt[:, gc0 + o:gc0 + o + cols]
                                .unsqueeze(2).to_broadcast((128, cols, 8)),
                            A.is_equal)
                        nc.vector.tensor_tensor(
                            s83, s83, G3[:, :, 1 + C:1 + C + 8], A.mult)
                        nc.vector.tensor_reduce(
                            sjsel[:, o:o + cols], s83, AX.X, A.add)
                    lnf = pbs.tile([128, cw], f32, tag="lnf")
                    nc.scalar.activation(lnf[:], siF[:, gc0:gc1], AF.Ln)
                    calp = pbs.tile([128, cw], f32, tag="calp")
                    nc.vector.scalar_tensor_tensor(
                        calp[:], lnf[:], 2.0, sjsel[:], A.mult, A.add)
                    nc.vector.scalar_tensor_tensor(
                        calp[:], calp[:], 0.2, calp[:], A.mult, A.max)
                    nc.vector.scalar_tensor_tensor(
                        calp[:], calp[:], 80.0, maskt[:, gc0:gc1],
                        A.min, A.add)
                    nc.scalar.activation(warr[:, gc0:gc1], calp[:], AF.Exp)
                    # weighting + z matmuls + per-block epilogue
                    for b in range(b0, b1):
                        zps = zpp.tile([128, 1 + C], f32, tag="zps")
                        first = True
                        nt = [(g, k) for g in range(NG)
                              for k in range(int(tiles[b, g]))]
                        for (g, k) in nt:
                            gc = int(call_off[w, g]) + colmap[(b, g, k)]
                            c = colmap[(b, g, k)]
                            G3 = stgs[g][:].rearrange("p (c e) -> p c e",
                                                      e=ROWW)
                            whj = pbs.tile([128, 1 + C], dt.bfloat16,
                                           tag="whj")
                            if gc % 2 == 0:
                                nc.scalar.activation(
                                    whj[:], G3[:, c, 0:1 + C], AF.Copy,
                                    scale=warr[:, gc:gc + 1])
                            else:
                                nc.vector.tensor_scalar_mul(
                                    whj[:], G3[:, c, 0:1 + C],
                                    warr[:, gc:gc + 1])
                            nc.tensor.matmul(zps[:], sfacs[(b, g, k)][:],
                                             whj[:], start=first,
                                             stop=((g, k) == nt[-1]))
                            first = False
                        rec = pbs.tile([128, 1], f32, tag="rec")
                        nc.vector.tensor_scalar(rec[:], zps[:, 0:1], EPS,
                                                None, A.add)
                        nc.vector.reciprocal(rec[:], rec[:])
                        zn = pbs.tile([128, C], f32, tag="zn")
                        nc.vector.tensor_scalar_mul(zn[:], zps[:, 1:1 + C],
                                                    rec[:])
                        ztp = pbp.tile([C, 128], f32, tag="ztp")
                        nc.tensor.transpose(ztp[:], zn[:], ident[:])
                        if b % 4 == 0:
                            zt4 = pb.tile([C, 4 * TB], f32, tag="zt4")
                        nc.scalar.activation(
                            zt4[:C, (b % 4) * TB:(b % 4 + 1) * TB], ztp[:],
                            AF.Copy)
                        if b % 4 == 3 or b == NBLK - 1:
                            bb0 = (b // 4) * 4
                            nc.sync.dma_start(
                                zT[:C, bb0 * TB:(b + 1) * TB],
                                zt4[:C, 0:(b + 1 - bb0) * TB])

            if _stop == f"B{lay}":
                return
            # ---------------- Phase C: tail ----------------
            with tc.tile_pool(name="pc", bufs=3) as pc, \
                 tc.tile_pool(name="pcw", bufs=1) as pcw, \
                 tc.tile_pool(name="pcp", bufs=1, space="PSUM") as pcp, \
                 tc.tile_pool(name="pcq", bufs=1, space="PSUM") as pcq:
                wq = pcw.tile([C, 8 * C], f32, tag="wq")
                nc.sync.dma_start(wq[:], Wq[:])
                wk = pcw.tile([C, 8 * C], f32, tag="wk")
                nc.sync.dma_start(wk[:], Wk[:])
                wv = pcw.tile([C, 8 * C], f32, tag="wv")
                nc.sync.dma_start(wv[:], Wv[:])
                ws = pcw.tile([C, C], f32, tag="ws")
                nc.sync.dma_start(ws[:], Ws[:])
                for k in range((Nc + 127) // 128):
                    n0 = k * 128
                    nn = min(128, Nc - n0)
                    ztc = pc.tile([C, 1024], f32, tag="ztc")
                    nc.sync.dma_start(ztc[:C, 0:nn * 8],
                                      zT[:C, n0 * 8:(n0 + nn) * 8])
                    zt3 = ztc[:C, 0:nn * 8].rearrange("c (n r) -> c r n", r=8)
                    kps = pcp.tile([C, 128], f32, tag="kps")
                    vps = pcp.tile([C, 128], f32, tag="vps")
                    for r in range(8):
                        nc.tensor.matmul(kps[:C, :nn],
                                         wk[:, r * C:(r + 1) * C],
                                         zt3[:, r, :], start=(r == 0),
                                         stop=(r == 7))
                    for r in range(8):
                        nc.tensor.matmul(vps[:C, :nn],
                                         wv[:, r * C:(r + 1) * C],
                                         zt3[:, r, :], start=(r == 0),
                                         stop=(r == 7))
                    ks = pc.tile([C, 128], f32, tag="ks")
                    nc.scalar.activation(ks[:C, :nn], kps[:C, :nn], AF.Copy)
                    pps = pcp.tile([8, 128], f32, tag="pps")
                    for r in range(8):
                        qps = pcq.tile([C, 128], f32, tag="qps")
                        nc.tensor.matmul(qps[:C, :nn],
                                         wq[:, r * C:(r + 1) * C],
                                         zt3[:, r, :], start=True, stop=True)
                        tmp = pc.tile([C, 128], f32, tag="tmp")
                        nc.vector.tensor_tensor(tmp[:C, :nn], qps[:C, :nn],
                                                ks[:C, :nn], A.mult)
                        nc.tensor.matmul(pps[:8, :nn],
                                         sel64[:C, r * 8:(r + 1) * 8],
                                         tmp[:C, :nn], start=(r == 0),
                                         stop=(r == 7))
                    psis = pc.tile([8, 128], f32, tag="psis")
                    nc.scalar.activation(psis[:8, :nn], pps[:8, :nn], AF.Copy)
                    ptp = pcq.tile([128, 8], f32, tag="ptp")
                    nc.tensor.transpose(ptp[:nn, :], psis[:8, :nn],
                                        ident[0:8, 0:8])
                    psiT = pc.tile([128, 8], f32, tag="psiT")
                    nc.scalar.activation(psiT[:nn, :], ptp[:nn, :], AF.Copy)
                    vs = pc.tile([C, 128], f32, tag="vs")
                    nc.scalar.activation(vs[:C, :nn], vps[:C, :nn], AF.Copy)
                    vtp = pcq.tile([128, C], f32, tag="vtp")
                    nc.tensor.transpose(vtp[:nn, :], vs[:C, :nn],
                                        ident[0:C, 0:C])
                    vsum = pc.tile([128, C], f32, tag="vsum")
                    nc.scalar.activation(vsum[:nn, :], vtp[:nn, :], AF.Copy)
                    bps = pcq.tile([128, C], f32, tag="bps")
                    nc.tensor.matmul(bps[:nn, :], hT[:C, n0:n0 + nn], ws[:],
                                     start=True, stop=True)
                    # delta
                    g2 = pc.tile([128, 8 * C], f32, tag="g2")
                    g23 = g2[:nn, :].rearrange("p (r c) -> p r c", r=8)
                    nc.vector.tensor_tensor(
                        g23,
                        vsum[:nn, :].unsqueeze(1).to_broadcast((nn, 8, C)),
                        psiT[:nn, :].unsqueeze(2).to_broadcast((nn, 8, C)),
                        A.mult)
                    nc.vector.tensor_tensor(
                        g23, g23,
                        bps[:nn, :].unsqueeze(1).to_broadcast((nn, 8, C)),
                        A.add)
                    m8 = pc.tile([128, 8], f32, tag="m8")
                    nc.vector.tensor_reduce(m8[:nn, :], g23, AX.X, A.max)
                    nc.vector.tensor_scalar_mul(m8[:nn, :], m8[:nn, :], -1.0)
                    ssum = pc.tile([128, 8], f32, tag="ssum")
                    for r in range(8):
                        nc.scalar.activation(
                            g2[:nn, r * C:(r + 1) * C],
                            g2[:nn, r * C:(r + 1) * C], AF.Exp,
                            bias=m8[:nn, r:r + 1],
                            accum_out=ssum[:nn, r:r + 1])
                    nc.vector.reciprocal(ssum[:nn, :], ssum[:nn, :])
                    for r in range(8):
                        nc.scalar.activation(
                            g2[:nn, r * C:(r + 1) * C],
                            g2[:nn, r * C:(r + 1) * C], AF.Copy,
                            scale=ssum[:nn, r:r + 1])
                    outc = pc.tile([128, C], f32, tag="outc")
                    nc.vector.tensor_reduce(
                        outc[:nn, :],
                        g2[:nn, :].rearrange("p (r c) -> p c r", r=8),
                        AX.X, A.add)
                    if lay == 1:
                        otp = pcq.tile([C, 128], f32, tag="vtp")
                        nc.tensor.transpose(otp[:C, :nn], outc[:nn, :],
                                            ident[0:nn, 0:nn])
                        ot = pc.tile([C, 128], f32, tag="ot")
                        nc.scalar.activation(ot[:C, :nn], otp[:C, :nn],
                                             AF.Copy)
                        nc.sync.dma_start(H['o1T'][:, n0:n0 + nn],
                                          ot[:C, :nn])
                    else:
                        m1 = pc.tile([128, 1], f32, tag="m1")
                        nc.vector.tensor_reduce(m1[:nn, :], outc[:nn, :],
                                                AX.X, A.max)
                        nc.vector.tensor_scalar_mul(m1[:nn, :], m1[:nn, :],
                                                    -1.0)
                        sc = pc.tile([128, C], f32, tag="sc")
                        s1 = pc.tile([128, 1], f32, tag="s1")
                        nc.scalar.activation(sc[:nn, :], outc[:nn, :], AF.Exp,
                                             bias=m1[:nn, :],
                                             accum_out=s1[:nn, :])
                        lns = pc.tile([128, 1], f32, tag="lns")
                        nc.scalar.activation(lns[:nn, :], s1[:nn, :], AF.Ln)
                        res = pc.tile([128, C], f32, tag="res")
                        nc.vector.scalar_tensor_tensor(
                            res[:nn, :], outc[:nn, :], m1[:nn, :],
                            lns[:nn, :].to_broadcast((nn, C)),
                            A.add, A.subtract)
                        nc.sync.dma_start(H['out'][n0:n0 + nn, :],
                                          res[:nn, :])
            if _stop == f"C{lay}":
                return
            if lay == 1:
                nc.gpsimd.collective_compute(
                    "AllGather", A.bypass,
                    replica_groups=[list(range(NCORES))],
                    ins=[H['o1T'][:]],
                    outs=[H['x2T'][:]])


def kernel(**inputs):
    import concourse.bass as bass
    import concourse.bacc as bacc
    import concourse.mybir as mybir
    import concourse.tile as tile
    from concourse.bass_utils import run_bass_kernel_spmd

    ins = {k: np.asarray(v) for k, v in inputs.items()}
    tabs, meta = _host_prep(ins['edge_index'], ins['edge_type'])
    ncols = meta['ncols']
    gcols = meta['gcols']

    f32 = mybir.dt.float32
    i16 = mybir.dt.int16
    nc = bacc.Bacc("TRN2", target_bir_lowering=False, debug=False,
                   num_devices=NCORES)

    def din(name, shape, dtype=f32):
        return nc.dram_tensor(name, list(shape), dtype,
                              kind="ExternalInput").ap()

    H = {}
    H['xT'] = din("xT", [128, N])
    H['xTloc'] = din("xTloc", [128, Nc])
    H['iota128'] = din("iota128", [128, 128])
    H['iota8'] = din("iota8", [128, 8])
    H['ident'] = din("ident", [128, 128])
    H['sel64'] = din("sel64", [128, 64])
    H['tcol'] = din("tcol", [128, ncols])
    H['etv'] = din("etv", [128, ncols])
    H['mask'] = din("mask", [128, ncols])
    H['idx'] = [din(f"idx{g}", [128, gcols[g] * 8], i16) for g in range(NG)]
    for l, c in ((1, C1), (2, C2)):
        H[f'Wcat{l}'] = din(f"Wcat{l}", [128, c + 8])
        H[f'attiT{l}'] = din(f"attiT{l}", [c, 8])
        H[f'Wq{l}'] = din(f"Wq{l}", [c, 8 * c])
        H[f'Wk{l}'] = din(f"Wk{l}", [c, 8 * c])
        H[f'Wv{l}'] = din(f"Wv{l}", [c, 8 * c])
        H[f'Ws{l}'] = din(f"Ws{l}", [c, c])
    H['hx1'] = nc.dram_tensor("hx1", [N, ROW1], f32).ap()
    H['hx2'] = nc.dram_tensor("hx2", [N, ROW2], f32).ap()
    H['zT1'] = nc.dram_tensor("zT1", [C1, RNc_pad], f32).ap()
    H['zT2'] = nc.dram_tensor("zT2", [C2, RNc_pad], f32).ap()
    H['F1'] = nc.dram_tensor("F1", [RNc_pad], f32).ap()
    H['F2'] = nc.dram_tensor("F2", [RNc_pad], f32).ap()
    H['o1T'] = nc.dram_tensor("o1T", [128, Nc], f32).ap()
    H['x2T'] = nc.dram_tensor("x2T", [NCORES * 128, Nc], f32,
                              addr_space="Shared").ap()
    H['out'] = nc.dram_tensor("out", [Nc, C2], f32,
                              kind="ExternalOutput").ap()

    with tile.TileContext(nc) as tc:
        _emit(nc, tc, bass, mybir, meta, H)
    nc.compile()

    # host-side constant inputs
    x = ins['x'].astype(np.float32)
    iota128 = np.broadcast_to(np.arange(128, dtype=np.float32), (128, 128))
    iota8 = np.broadcast_to(np.arange(8, dtype=np.float32), (128, 8))
    ident = np.eye(128, dtype=np.float32)
    sel64 = np.zeros((128, 64), np.float32)
    for r in range(8):
        sel64[:, r * 8 + r] = 1.0

    common = dict(
        xT=np.ascontiguousarray(x.T),
        iota128=np.ascontiguousarray(iota128),
        iota8=np.ascontiguousarray(iota8),
        ident=ident, sel64=sel64,
    )
    for l, c in ((1, C1), (2, C2)):
        att = ins[f'att{l}'].astype(np.float32)
        Wn = ins[f'Wn{l}'].astype(np.float32)
        common[f'Wcat{l}'] = np.ascontiguousarray(
            np.concatenate([Wn, Wn @ att[:, c:].T], axis=1))
        common[f'attiT{l}'] = np.ascontiguousarray(att[:, :c].T)
        for nm in ('Wq', 'Wk', 'Wv'):
            W = ins[f'{nm}{l}'].astype(np.float32)
            common[f'{nm}{l}'] = np.ascontiguousarray(
                W.transpose(1, 0, 2).reshape(c, 8 * c))
        common[f'Ws{l}'] = ins[f'Ws{l}'].astype(np.float32)

    in_maps = []
    for m in range(NCORES):
        im = dict(common)
        im['xTloc'] = np.ascontiguousarray(x.T[:, m * Nc:(m + 1) * Nc])
        im['tcol'] = tabs[m]['tcol']
        im['etv'] = tabs[m]['etv']
        im['mask'] = tabs[m]['mask']
        for g in range(NG):
            im[f'idx{g}'] = tabs[m]['idx'][g]
        in_maps.append(im)

    global _LAST_BUILD, LAST_EXEC_NS
    _LAST_BUILD = (nc, in_maps)
    res = run_bass_kernel_spmd(nc, in_maps, list(range(NCORES)))
    LAST_EXEC_NS = res.exec_time_ns
    out = np.concatenate([res.results[m]['out'] for m in range(NCORES)], 0)
    return out.astype(np.float32)


LAST_EXEC_NS = None
_LAST_BUILD = None


# revision 12
# speedup vs baseline: 6.0333x; 1.1985x over previous
"""BRGCN (2-layer relational GCN with bilinear attention) on 8 TRN2 NeuronCores.

Sharding: edges are sharded by dst node (6250 nodes/core) so the
(relation, dst) segment softmax and the z scatter are fully core-local; node
features are replicated. Per core the edges are sorted by t = dst_local*8+rel,
binned into 391 blocks of 128 t-rows and padded into 128-edge K-tiles (tile
counts uniform across cores -> one SPMD program; the kernel is compiled per
call so all binning is static). The z scatter is a one-hot matmul with
Sfac[e,t] = (iota[t]==t_e)*F[t], F[t]=exp(si[t]/2); the fused accum_out of that
op delivers the per-edge dst attention term si, and F cancels in the z/d
normalization. Per-edge src features come from one dma_gather of 768B rows
[1 | h | sj | pad]. The tail (per-relation q/k/v, psi, channel softmax) is
node-parallel; one AllGather bridges layer1 -> layer2.
"""
import numpy as np

R = 8
N = 50000
E = 800000
NCORES = 8
Nc = N // NCORES            # 6250
TB = 128
RNc = R * Nc                # 50000
RNc_pad = ((RNc + TB - 1) // TB) * TB   # 50048
NBLK = RNc_pad // TB        # 391
SPLITS = [0, 16672, 33344, N]
NG = 3
C1, C2 = 128, 64
ROW1, ROW2 = 192, 128       # gathered row length in f32 words
EPS = 1e-30
WBLK = 8                    # blocks per gather window
NW = (NBLK + WBLK - 1) // WBLK


def _host_prep(edge_index, edge_type):
    src = edge_index[0].astype(np.int64)
    dst = edge_index[1].astype(np.int64)
    et = edge_type.astype(np.int64)
    percore = []
    counts = np.zeros((NCORES, NBLK, NG), np.int64)
    for m in range(NCORES):
        sel = (dst // Nc) == m
        s, d, r = src[sel], dst[sel], et[sel]
        t = (d - m * Nc) * 8 + r
        blk = t // TB
        grp = np.digitize(s, SPLITS[1:NG])
        order = np.lexsort((t, grp, blk))
        s, t, r, blk, grp = s[order], t[order], r[order], blk[order], grp[order]
        percore.append((s, t, r, blk, grp))
        for g in range(NG):
            np.add.at(counts[m, :, g], blk[grp == g], 1)
    tiles = np.maximum(-(-counts.max(0) // 128), 1)   # [NBLK, NG]

    # column maps, shared across cores
    colmap = {}
    call_cols = np.zeros((NW, NG), np.int64)
    for w in range(NW):
        for g in range(NG):
            c = 0
            for b in range(w * WBLK, min((w + 1) * WBLK, NBLK)):
                for k in range(int(tiles[b, g])):
                    colmap[(b, g, k)] = c
                    c += 1
            call_cols[w, g] = c
    call_off = np.zeros((NW, NG), np.int64)
    acc = 0
    for w in range(NW):
        for g in range(NG):
            call_off[w, g] = acc
            acc += call_cols[w, g]
    ncols = int(acc)
    gcols = [int(call_cols[:, g].sum()) for g in range(NG)]
    gcol_off = np.zeros((NW, NG), np.int64)
    for g in range(NG):
        a = 0
        for w in range(NW):
            gcol_off[w, g] = a
            a += call_cols[w, g]

    tabs = []
    for m in range(NCORES):
        s, t, r, blk, grp = percore[m]
        tcol = np.zeros(ncols * 128, np.float32)
        etv = np.zeros(ncols * 128, np.float32)
        mask = np.full(ncols * 128, -1e30, np.float32)
        idxs = [np.zeros(gcols[g] * 128, np.int64) for g in range(NG)]
        for b in range(NBLK):
            w = b // WBLK
            for g in range(NG):
                bm = (blk == b) & (grp == g)
                cnt = int(bm.sum())
                c0 = colmap[(b, g, 0)]
                o = (int(call_off[w, g]) + c0) * 128
                tcol[o:o + cnt] = (t[bm] - b * TB).astype(np.float32)
                etv[o:o + cnt] = r[bm].astype(np.float32)
                mask[o:o + cnt] = 0.0
                io = (int(gcol_off[w, g]) + c0) * 128
                idxs[g][io:io + cnt] = s[bm] - SPLITS[g]

        def wrap16(a):
            ar = a.reshape(-1, 16).T
            return np.ascontiguousarray(np.tile(ar, (8, 1))).astype(np.int16)

        tabs.append(dict(
            tcol=np.ascontiguousarray(tcol.reshape(-1, 128).T),
            etv=np.ascontiguousarray(etv.reshape(-1, 128).T),
            mask=np.ascontiguousarray(mask.reshape(-1, 128).T),
            idx=[wrap16(idxs[g]) for g in range(NG)],
        ))
    meta = dict(tiles=tiles, colmap=colmap, call_cols=call_cols,
                call_off=call_off, gcol_off=gcol_off, ncols=ncols, gcols=gcols)
    return tabs, meta


def _emit(nc, tc, bass, mybir, meta, H):
    import os
    _stop = os.environ.get("BRGCN_STOP", "")
    dt = mybir.dt
    f32 = dt.float32
    A = mybir.AluOpType
    AF = mybir.ActivationFunctionType
    AX = mybir.AxisListType
    tiles = meta['tiles']; colmap = meta['colmap']
    call_cols = meta['call_cols']; call_off = meta['call_off']
    gcol_off = meta['gcol_off']; ncols = meta['ncols']

    with tc.tile_pool(name="persist", bufs=1) as pp:
        iota128 = pp.tile([128, 128], f32, tag="iota128")
        nc.sync.dma_start(iota128[:], H['iota128'][:])
        iota8 = pp.tile([128, 8], f32, tag="iota8")
        nc.sync.dma_start(iota8[:], H['iota8'][:])
        ones1 = pp.tile([1, 128], f32, tag="ones1")
        nc.vector.memset(ones1[:], 1.0)
        ident = pp.tile([128, 128], f32, tag="ident")
        nc.sync.dma_start(ident[:], H['ident'][:])
        sel64 = pp.tile([128, 64], f32, tag="sel64")
        nc.sync.dma_start(sel64[:], H['sel64'][:])
        tcolt = pp.tile([128, ncols], f32, tag="tcolt")
        nc.sync.dma_start(tcolt[:], H['tcol'][:])
        etvt = pp.tile([128, ncols], f32, tag="etvt")
        nc.sync.dma_start(etvt[:], H['etv'][:])
        maskt = pp.tile([128, ncols], f32, tag="maskt")
        nc.sync.dma_start(maskt[:], H['mask'][:])
        siF = pp.tile([128, ncols], f32, tag="siF")
        warr = pp.tile([128, ncols], f32, tag="warr")
        hT1 = pp.tile([C1, Nc], f32, tag="hT1")
        hT2 = pp.tile([C2, Nc], f32, tag="hT2")

        for lay in (1, 2):
            C = C1 if lay == 1 else C2
            ROWW = ROW1 if lay == 1 else ROW2
            hT = hT1 if lay == 1 else hT2
            hx = H['hx1'] if lay == 1 else H['hx2']
            zT = H['zT1'] if lay == 1 else H['zT2']
            Fd = H['F1'] if lay == 1 else H['F2']
            Wcat = H['Wcat1'] if lay == 1 else H['Wcat2']
            attiT = H['attiT1'] if lay == 1 else H['attiT2']
            Wq = H['Wq1'] if lay == 1 else H['Wq2']
            Wk = H['Wk1'] if lay == 1 else H['Wk2']
            Wv = H['Wv1'] if lay == 1 else H['Wv2']
            Ws = H['Ws1'] if lay == 1 else H['Ws2']
            CIN = 128

            # source tiles for phase A (full N in <=128-node pieces)
            if lay == 1:
                abatches = [(H['xT'][:, k0:k0 + min(512, N - k0)], k0,
                             min(512, N - k0)) for k0 in range(0, N, 512)]
                xt_loc = H['xTloc']
            else:
                abatches = []
                for g in range(NCORES):
                    for k0 in range(0, Nc, 512):
                        tot = min(512, Nc - k0)
                        abatches.append(
                            (H['x2T'][g * 128:(g + 1) * 128, k0:k0 + tot],
                             g * Nc + k0, tot))
                xt_loc = H['o1T'][:, :]

            # ---------------- Phase A ----------------
            with tc.tile_pool(name="pa", bufs=3) as pa, \
                 tc.tile_pool(name="pap", bufs=2, space="PSUM") as pap:
                wcat = pa.tile([CIN, C + 8], f32, tag="wcat")
                nc.sync.dma_start(wcat[:], Wcat[:])
                W = 1 + C + 8
                for (src_ap, n0, tot) in abatches:
                    nt = (tot + 127) // 128
                    xt_sb = pa.tile([CIN, 512], f32, tag="xt")
                    nc.sync.dma_start(xt_sb[:, :tot], src_ap)
                    stg = pa.tile([128, 4 * W], f32, tag="stg")
                    for j in range(nt):
                        nn = min(128, tot - j * 128)
                        hps = pap.tile([128, C + 8], f32, tag="hps")
                        nc.tensor.matmul(hps[:nn, :],
                                         xt_sb[:, j * 128:j * 128 + nn],
                                         wcat[:], start=True, stop=True)
                        nc.vector.memset(stg[:nn, j * W:j * W + 1], 1.0)
                        nc.scalar.activation(stg[:nn, j * W + 1:(j + 1) * W],
                                             hps[:nn, :], AF.Copy)
                    nfull = tot // 128
                    rem = tot - nfull * 128
                    if nfull:
                        nc.sync.dma_start(
                            bass.AP(hx.tensor, n0 * ROWW,
                                    [[ROWW, 128], [ROWW * 128, nfull],
                                     [1, W]]),
                            stg[:, 0:nfull * W].rearrange(
                                "p (j w) -> p j w", w=W))
                    if rem:
                        nc.sync.dma_start(
                            hx[n0 + nfull * 128:n0 + tot, 0:W],
                            stg[:rem, nfull * W:(nfull + 1) * W])
                # hT_local
                for k in range((Nc + 127) // 128):
                    n0 = k * 128
                    nn = min(128, Nc - n0)
                    xt_sb = pa.tile([CIN, 128], f32, tag="xt")
                    nc.sync.dma_start(xt_sb[:, :nn], xt_loc[:, n0:n0 + nn])
                    hps = pap.tile([128, 128], f32, tag="hps2")
                    nc.tensor.matmul(hps[:C, :nn], wcat[:, 0:C], xt_sb[:, :nn],
                                     start=True, stop=True)
                    nc.scalar.activation(hT[:C, n0:n0 + nn], hps[:C, :nn],
                                         AF.Copy)
                # F table
                atti = pa.tile([C, 8], f32, tag="atti")
                nc.sync.dma_start(atti[:], attiT[:])
                FT = pa.tile([8, Nc], f32, tag="FT")
                for k in range((Nc + 127) // 128):
                    n0 = k * 128
                    nn = min(128, Nc - n0)
                    sps = pap.tile([8, 128], f32, tag="sps")
                    nc.tensor.matmul(sps[:8, :nn], atti[:C, :],
                                     hT[:C, n0:n0 + nn], start=True, stop=True)
                    nc.scalar.activation(FT[:, n0:n0 + nn], sps[:8, :nn],
                                         AF.Exp, scale=0.5)
                nc.sync.dma_start(bass.AP(Fd.tensor, 0, [[1, 8], [8, Nc]]),
                                  FT[:, :Nc])
                nc.sync.dma_start(Fd[RNc:RNc_pad],
                                  ones1[0:1, 0:RNc_pad - RNc])

            if _stop == f"A{lay}":
                return
            # ---------------- Phase B: edges ----------------
            with tc.tile_pool(name="pb", bufs=2) as pb, \
                 tc.tile_pool(name="pbs", bufs=4) as pbs, \
                 tc.tile_pool(name="sfp", bufs=4 * WBLK + 8) as sfp, \
                 tc.tile_pool(name="pbp", bufs=2, space="PSUM") as pbp, \
                 tc.tile_pool(name="zpp", bufs=3, space="PSUM") as zpp:
                for w in range(NW):
                    b0 = w * WBLK
                    b1 = min(b0 + WBLK, NBLK)
                    stgs = []
                    for g in range(NG):
                        cols = int(call_cols[w, g])
                        gt = pb.tile([128, cols * ROWW], f32, tag=f"G{g}")
                        io = int(gcol_off[w, g])
                        idxt = pbs.tile([128, cols * 8], dt.int16, tag=f"ix{g}")
                        nc.sync.dma_start(
                            idxt[:], H['idx'][g][:, io * 8:(io + cols) * 8])
                        nc.gpsimd.dma_gather(
                            gt[:].rearrange("p (c e) -> p c e", e=ROWW),
                            hx[SPLITS[g]:SPLITS[g + 1], 0:ROWW],
                            idxt[:], cols * 128, cols * 128, ROWW,
                            elem_step=ROWW)
                        stgs.append(gt)
                    fsb = pbs.tile([1, WBLK * TB], f32, tag="fsb")
                    nc.sync.dma_start(fsb[0:1, 0:(b1 - b0) * TB],
                                      Fd[b0 * TB:b1 * TB])
                    sfacs = {}
                    for b in range(b0, b1):
                        frep = pbp.tile([128, TB], f32, tag="frep")
                        nc.tensor.matmul(
                            frep[:], ones1[:],
                            fsb[0:1, (b - b0) * TB:(b - b0 + 1) * TB],
                            start=True, stop=True)
                        for g in range(NG):
                            for k in range(int(tiles[b, g])):
                                gc = int(call_off[w, g]) + colmap[(b, g, k)]
                                sf = sfp.tile([128, TB], f32, tag="sfac")
                                nc.vector.scalar_tensor_tensor(
                                    sf[:], iota128[:], tcolt[:, gc:gc + 1],
                                    frep[:], A.is_equal, A.mult,
                                    accum_out=siF[:, gc:gc + 1])
                                sfacs[(b, g, k)] = sf
                    # alpha / w for the whole window
                    gc0 = int(call_off[w, 0])
                    gc1 = int(call_off[w, NG - 1] + call_cols[w, NG - 1])
                    cw = gc1 - gc0
                    sjsel = pbs.tile([128, cw], f32, tag="sjsel")
                    for g in range(NG):
                        cols = int(call_cols[w, g])
                        o = int(call_off[w, g]) - gc0
                        G3 = stgs[g][:].rearrange("p (c e) -> p c e", e=ROWW)
                        s8 = pbs.tile([128, cols * 8], f32, tag="s8")
                        s83 = s8[:].rearrange("p (c e) -> p c e", e=8)
                        nc.vector.tensor_tensor(
                            s83,
                            iota8[:].unsqueeze(1).to_broadcast((128, cols, 8)),
                            etvt[:, gc0 + o:gc0 + o + cols]
                                .unsqueeze(2).to_broadcast((128, cols, 8)),
                            A.is_equal)
                        nc.vector.tensor_tensor(
                            s83, s83, G3[:, :, 1 + C:1 + C + 8], A.mult)
                        nc.vector.tensor_reduce(
                            sjsel[:, o:o + cols], s83, AX.X, A.add)
                    lnf = pbs.tile([128, cw], f32, tag="lnf")
                    nc.scalar.activation(lnf[:], siF[:, gc0:gc1], AF.Ln)
                    calp = pbs.tile([128, cw], f32, tag="calp")
                    nc.vector.scalar_tensor_tensor(
                        calp[:], lnf[:], 2.0, sjsel[:], A.mult, A.add)
                    nc.vector.scalar_tensor_tensor(
                        calp[:], calp[:], 0.2, calp[:], A.mult, A.max)
                    nc.vector.scalar_tensor_tensor(
                        calp[:], calp[:], 80.0, maskt[:, gc0:gc1],
                        A.min, A.add)
                    nc.scalar.activation(warr[:, gc0:gc1], calp[:], AF.Exp)
                    # weighting + z matmuls + per-block epilogue
                    for b in range(b0, b1):
                        zps = zpp.tile([128, 1 + C], f32, tag="zps")
                        first = True
                        nt = [(g, k) for g in range(NG)
                              for k in range(int(tiles[b, g]))]
                        for (g, k) in nt:
                            gc = int(call_off[w, g]) + colmap[(b, g, k)]
                            c = colmap[(b, g, k)]
                            G3 = stgs[g][:].rearrange("p (c e) -> p c e",
                                                      e=ROWW)
                            whj = pbs.tile([128, 1 + C], f32, tag="whj")
                            if gc % 2 == 0:
                                nc.scalar.activation(
                                    whj[:], G3[:, c, 0:1 + C], AF.Copy,
                                    scale=warr[:, gc:gc + 1])
                            else:
                                nc.vector.tensor_scalar_mul(
                                    whj[:], G3[:, c, 0:1 + C],
                                    warr[:, gc:gc + 1])
                            nc.tensor.matmul(zps[:], sfacs[(b, g, k)][:],
                                             whj[:], start=first,
                                             stop=((g, k) == nt[-1]))
                            first = False
                        rec = pbs.tile([128, 1], f32, tag="rec")
                        nc.vector.tensor_scalar(rec[:], zps[:, 0:1], EPS,
                                                None, A.add)
                        nc.vector.reciprocal(rec[:], rec[:])
                        zn = pbs.tile([128, C], f32, tag="zn")
                        nc.vector.tensor_scalar_mul(zn[:], zps[:, 1:1 + C],
                                                    rec[:])
                        ztp = pbp.tile([C, 128], f32, tag="ztp")
                        nc.tensor.transpose(ztp[:], zn[:], ident[:])
                        if b % 4 == 0:
                            zt4 = pb.tile([C, 4 * TB], f32, tag="zt4")
                        nc.scalar.activation(
                            zt4[:C, (b % 4) * TB:(b % 4 + 1) * TB], ztp[:],
                            AF.Copy)
                        if b % 4 == 3 or b == NBLK - 1:
                            bb0 = (b // 4) * 4
                            nc.sync.dma_start(
                                zT[:C, bb0 * TB:(b + 1) * TB],
                                zt4[:C, 0:(b + 1 - bb0) * TB])

            if _stop == f"B{lay}":
                return
            # ---------------- Phase C: tail ----------------
            with tc.tile_pool(name="pc", bufs=3) as pc, \
                 tc.tile_pool(name="pcw", bufs=1) as pcw, \
                 tc.tile_pool(name="pcp", bufs=1, space="PSUM") as pcp, \
                 tc.tile_pool(name="pcq", bufs=1, space="PSUM") as pcq:
                wq = pcw.tile([C, 8 * C], f32, tag="wq")
                nc.sync.dma_start(wq[:], Wq[:])
                wk = pcw.tile([C, 8 * C], f32, tag="wk")
                nc.sync.dma_start(wk[:], Wk[:])
                wv = pcw.tile([C, 8 * C], f32, tag="wv")
                nc.sync.dma_start(wv[:], Wv[:])
                ws = pcw.tile([C, C], f32, tag="ws")
                nc.sync.dma_start(ws[:], Ws[:])
                for k in range((Nc + 127) // 128):
                    n0 = k * 128
                    nn = min(128, Nc - n0)
                    ztc = pc.tile([C, 1024], f32, tag="ztc")
                    nc.sync.dma_start(ztc[:C, 0:nn * 8],
                                      zT[:C, n0 * 8:(n0 + nn) * 8])
                    zt3 = ztc[:C, 0:nn * 8].rearrange("c (n r) -> c r n", r=8)
                    kps = pcp.tile([C, 128], f32, tag="kps")
                    vps = pcp.tile([C, 128], f32, tag="vps")
                    for r in range(8):
                        nc.tensor.matmul(kps[:C, :nn],
                                         wk[:, r * C:(r + 1) * C],
                                         zt3[:, r, :], start=(r == 0),
                                         stop=(r == 7))
                    for r in range(8):
                        nc.tensor.matmul(vps[:C, :nn],
                                         wv[:, r * C:(r + 1) * C],
                                         zt3[:, r, :], start=(r == 0),
                                         stop=(r == 7))
                    ks = pc.tile([C, 128], f32, tag="ks")
                    nc.scalar.activation(ks[:C, :nn], kps[:C, :nn], AF.Copy)
                    pps = pcp.tile([8, 128], f32, tag="pps")
                    for r in range(8):
                        qps = pcq.tile([C, 128], f32, tag="qps")
                        nc.tensor.matmul(qps[:C, :nn],
                                         wq[:, r * C:(r + 1) * C],
                                         zt3[:, r, :], start=True, stop=True)
                        tmp = pc.tile([C, 128], f32, tag="tmp")
                        nc.vector.tensor_tensor(tmp[:C, :nn], qps[:C, :nn],
                                                ks[:C, :nn], A.mult)
                        nc.tensor.matmul(pps[:8, :nn],
                                         sel64[:C, r * 8:(r + 1) * 8],
                                         tmp[:C, :nn], start=(r == 0),
                                         stop=(r == 7))
                    psis = pc.tile([8, 128], f32, tag="psis")
                    nc.scalar.activation(psis[:8, :nn], pps[:8, :nn], AF.Copy)
                    ptp = pcq.tile([128, 8], f32, tag="ptp")
                    nc.tensor.transpose(ptp[:nn, :], psis[:8, :nn],
                                        ident[0:8, 0:8])
                    psiT = pc.tile([128, 8], f32, tag="psiT")
                    nc.scalar.activation(psiT[:nn, :], ptp[:nn, :], AF.Copy)
                    vs = pc.tile([C, 128], f32, tag="vs")
                    nc.scalar.activation(vs[:C, :nn], vps[:C, :nn], AF.Copy)
                    vtp = pcq.tile([128, C], f32, tag="vtp")
                    nc.tensor.transpose(vtp[:nn, :], vs[:C, :nn],
                                        ident[0:C, 0:C])
                    vsum = pc.tile([128, C], f32, tag="vsum")
                    nc.scalar.activation(vsum[:nn, :], vtp[:nn, :], AF.Copy)
                    bps = pcq.tile([128, C], f32, tag="bps")
                    nc.tensor.matmul(bps[:nn, :], hT[:C, n0:n0 + nn], ws[:],
                                     start=True, stop=True)
                    # delta
                    g2 = pc.tile([128, 8 * C], f32, tag="g2")
                    g23 = g2[:nn, :].rearrange("p (r c) -> p r c", r=8)
                    nc.vector.tensor_tensor(
                        g23,
                        vsum[:nn, :].unsqueeze(1).to_broadcast((nn, 8, C)),
                        psiT[:nn, :].unsqueeze(2).to_broadcast((nn, 8, C)),
                        A.mult)
                    nc.vector.tensor_tensor(
                        g23, g23,
                        bps[:nn, :].unsqueeze(1).to_broadcast((nn, 8, C)),
                        A.add)
                    m8 = pc.tile([128, 8], f32, tag="m8")
                    nc.vector.tensor_reduce(m8[:nn, :], g23, AX.X, A.max)
                    nc.vector.tensor_scalar_mul(m8[:nn, :], m8[:nn, :], -1.0)
                    ssum = pc.tile([128, 8], f32, tag="ssum")
                    for r in range(8):
                        nc.scalar.activation(
                            g2[:nn, r * C:(r + 1) * C],
                            g2[:nn, r * C:(r + 1) * C], AF.Exp,
                            bias=m8[:nn, r:r + 1],
                            accum_out=ssum[:nn, r:r + 1])
                    nc.vector.reciprocal(ssum[:nn, :], ssum[:nn, :])
                    for r in range(8):
                        nc.scalar.activation(
                            g2[:nn, r * C:(r + 1) * C],
                            g2[:nn, r * C:(r + 1) * C], AF.Copy,
                            scale=ssum[:nn, r:r + 1])
                    outc = pc.tile([128, C], f32, tag="outc")
                    nc.vector.tensor_reduce(
                        outc[:nn, :],
                        g2[:nn, :].rearrange("p (r c) -> p c r", r=8),
                        AX.X, A.add)
                    if lay == 1:
                        otp = pcq.tile([C, 128], f32, tag="vtp")
                        nc.tensor.transpose(otp[:C, :nn], outc[:nn, :],
                                            ident[0:nn, 0:nn])
                        ot = pc.tile([C, 128], f32, tag="ot")
                        nc.scalar.activation(ot[:C, :nn], otp[:C, :nn],
                                             AF.Copy)
                        nc.sync.dma_start(H['o1T'][:, n0:n0 + nn],
                                          ot[:C, :nn])
                    else:
                        m1 = pc.tile([128, 1], f32, tag="m1")
                        nc.vector.tensor_reduce(m1[:nn, :], outc[:nn, :],
                                                AX.X, A.max)
                        nc.vector.tensor_scalar_mul(m1[:nn, :], m1[:nn, :],
                                                    -1.0)
                        sc = pc.tile([128, C], f32, tag="sc")
                        s1 = pc.tile([128, 1], f32, tag="s1")
                        nc.scalar.activation(sc[:nn, :], outc[:nn, :], AF.Exp,
                                             bias=m1[:nn, :],
                                             accum_out=s1[:nn, :])
                        lns = pc.tile([128, 1], f32, tag="lns")
                        nc.scalar.activation(lns[:nn, :], s1[:nn, :], AF.Ln)
                        res = pc.tile([128, C], f32, tag="res")
                        nc.vector.scalar_tensor_tensor(
                            res[:nn, :], outc[:nn, :], m1[:nn, :],
                            lns[:nn, :].to_broadcast((nn, C)),
                            A.add, A.subtract)
                        nc.sync.dma_start(H['out'][n0:n0 + nn, :],
                                          res[:nn, :])
            if _stop == f"C{lay}":
                return
            if lay == 1:
                nc.gpsimd.collective_compute(
                    "AllGather", A.bypass,
                    replica_groups=[list(range(NCORES))],
                    ins=[H['o1T'][:]],
                    outs=[H['x2T'][:]])


def kernel(**inputs):
    import concourse.bass as bass
    import concourse.bacc as bacc
    import concourse.mybir as mybir
    import concourse.tile as tile
    from concourse.bass_utils import run_bass_kernel_spmd

    ins = {k: np.asarray(v) for k, v in inputs.items()}
    tabs, meta = _host_prep(ins['edge_index'], ins['edge_type'])
    ncols = meta['ncols']
    gcols = meta['gcols']

    f32 = mybir.dt.float32
    i16 = mybir.dt.int16
    nc = bacc.Bacc("TRN2", target_bir_lowering=False, debug=False,
                   num_devices=NCORES)

    def din(name, shape, dtype=f32):
        return nc.dram_tensor(name, list(shape), dtype,
                              kind="ExternalInput").ap()

    H = {}
    H['xT'] = din("xT", [128, N])
    H['xTloc'] = din("xTloc", [128, Nc])
    H['iota128'] = din("iota128", [128, 128])
    H['iota8'] = din("iota8", [128, 8])
    H['ident'] = din("ident", [128, 128])
    H['sel64'] = din("sel64", [128, 64])
    H['tcol'] = din("tcol", [128, ncols])
    H['etv'] = din("etv", [128, ncols])
    H['mask'] = din("mask", [128, ncols])
    H['idx'] = [din(f"idx{g}", [128, gcols[g] * 8], i16) for g in range(NG)]
    for l, c in ((1, C1), (2, C2)):
        H[f'Wcat{l}'] = din(f"Wcat{l}", [128, c + 8])
        H[f'attiT{l}'] = din(f"attiT{l}", [c, 8])
        H[f'Wq{l}'] = din(f"Wq{l}", [c, 8 * c])
        H[f'Wk{l}'] = din(f"Wk{l}", [c, 8 * c])
        H[f'Wv{l}'] = din(f"Wv{l}", [c, 8 * c])
        H[f'Ws{l}'] = din(f"Ws{l}", [c, c])
    H['hx1'] = nc.dram_tensor("hx1", [N, ROW1], f32).ap()
    H['hx2'] = nc.dram_tensor("hx2", [N, ROW2], f32).ap()
    H['zT1'] = nc.dram_tensor("zT1", [C1, RNc_pad], f32).ap()
    H['zT2'] = nc.dram_tensor("zT2", [C2, RNc_pad], f32).ap()
    H['F1'] = nc.dram_tensor("F1", [RNc_pad], f32).ap()
    H['F2'] = nc.dram_tensor("F2", [RNc_pad], f32).ap()
    H['o1T'] = nc.dram_tensor("o1T", [128, Nc], f32).ap()
    H['x2T'] = nc.dram_tensor("x2T", [NCORES * 128, Nc], f32,
                              addr_space="Shared").ap()
    H['out'] = nc.dram_tensor("out", [Nc, C2], f32,
                              kind="ExternalOutput").ap()

    with tile.TileContext(nc) as tc:
        _emit(nc, tc, bass, mybir, meta, H)
    nc.compile()

    # host-side constant inputs
    x = ins['x'].astype(np.float32)
    iota128 = np.broadcast_to(np.arange(128, dtype=np.float32), (128, 128))
    iota8 = np.broadcast_to(np.arange(8, dtype=np.float32), (128, 8))
    ident = np.eye(128, dtype=np.float32)
    sel64 = np.zeros((128, 64), np.float32)
    for r in range(8):
        sel64[:, r * 8 + r] = 1.0

    common = dict(
        xT=np.ascontiguousarray(x.T),
        iota128=np.ascontiguousarray(iota128),
        iota8=np.ascontiguousarray(iota8),
        ident=ident, sel64=sel64,
    )
    for l, c in ((1, C1), (2, C2)):
        att = ins[f'att{l}'].astype(np.float32)
        Wn = ins[f'Wn{l}'].astype(np.float32)
        common[f'Wcat{l}'] = np.ascontiguousarray(
            np.concatenate([Wn, Wn @ att[:, c:].T], axis=1))
        common[f'attiT{l}'] = np.ascontiguousarray(att[:, :c].T)
        for nm in ('Wq', 'Wk', 'Wv'):
            W = ins[f'{nm}{l}'].astype(np.float32)
            common[f'{nm}{l}'] = np.ascontiguousarray(
                W.transpose(1, 0, 2).reshape(c, 8 * c))
        common[f'Ws{l}'] = ins[f'Ws{l}'].astype(np.float32)

    in_maps = []
    for m in range(NCORES):
        im = dict(common)
        im['xTloc'] = np.ascontiguousarray(x.T[:, m * Nc:(m + 1) * Nc])
        im['tcol'] = tabs[m]['tcol']
        im['etv'] = tabs[m]['etv']
        im['mask'] = tabs[m]['mask']
        for g in range(NG):
            im[f'idx{g}'] = tabs[m]['idx'][g]
        in_maps.append(im)

    global _LAST_BUILD, LAST_EXEC_NS
    _LAST_BUILD = (nc, in_maps)
    res = run_bass_kernel_spmd(nc, in_maps, list(range(NCORES)))
    LAST_EXEC_NS = res.exec_time_ns
    out = np.concatenate([res.results[m]['out'] for m in range(NCORES)], 0)
    return out.astype(np.float32)


LAST_EXEC_NS = None
_LAST_BUILD = None
